# revision 1
# baseline (speedup 1.0000x reference)
"""Trainium2 Bass kernel for nn_ComplexAttention (sparse single-query attention
over H history slots with complex-valued channels).

Key algebraic restructure (exact, not an approximation):
  reference computes   k = hist @ wk ; v = hist @ wv        (412 GFLOP)
  but scores[bt,h] = q[bt]·k[bt,h] = hist[bt,h]·(q @ wk^T)[bt]
  and  ctx[bt]     = sum_h attn[bt,h]*v[bt,h]
                   = (sum_h attn[bt,h]*hist[bt,h]) @ wv + bv   (sum attn = 1)
  so the two huge projection GEMMs collapse into two streaming passes over
  hist (one fused multiply-reduce for scores, one fused multiply-accumulate
  for the weighted mean) plus three small GEMMs (q, p=q@wk^T, ctx=m@wv).

Sharding: data-parallel over the flattened (B,T)=1024 rows, 128 rows/core
on 8 cores. Weights replicated. No collectives.

Per-core device program (hist resident in SBUF as [bt=128 part, H, 2D] bf16):
  q   = cur_cat @ wq                 (PE, lhsT = host-transposed cur_cat)
  qT  = transpose(q)                 (PE transposes, 8x 128x128)
  p   = q @ wk^T                     (PE, lhsT = qT, rhs = host-transposed wk)
  scores[:,h] = sum_c hist[:,h,:]*p  (DVE tensor_tensor_reduce, fused)
  attn = softmax(scores * conf/32)   (DVE + ACT exp)
  m   = sum_h attn[:,h]*hist[:,h,:]  (DVE scalar_tensor_tensor, fused mul-add)
  ctx = m @ wv                       (PE, lhsT = transpose(m))
  out = cur_cat + 0.1*ctx            (DVE fused, then DMA out)
"""

import numpy as np
import ml_dtypes

B, T, H, D = 4, 256, 32, 1024
C2 = 2 * D          # 2048
NCORES = 8
RPC = (B * T) // NCORES   # 128 rows (b,t) per core
P = 128

BF16 = ml_dtypes.bfloat16

_CACHE: dict = {}


def _build_program(has_bq: bool, has_bk: bool, has_bv: bool, m_bf16: bool):
    import concourse.bass as bass
    import concourse.mybir as mybir
    import concourse.tile as tile
    from concourse import bacc
    from concourse.masks import make_identity

    dt = mybir.dt
    f32, bf16 = dt.float32, dt.bfloat16
    mult = mybir.AluOpType.mult
    add = mybir.AluOpType.add
    Ax = mybir.AxisListType

    nc = bacc.Bacc(
        "TRN2",
        target_bir_lowering=False,
        debug=False,
        enable_asserts=False,
        num_devices=NCORES,
    )

    hist_d = nc.dram_tensor("hist", [RPC, H, C2], bf16, kind="ExternalInput").ap()
    curT_d = nc.dram_tensor("curT", [C2, RPC], bf16, kind="ExternalInput").ap()
    cur_d = nc.dram_tensor("cur", [RPC, C2], f32, kind="ExternalInput").ap()
    conf_d = nc.dram_tensor("conf", [RPC, 1], f32, kind="ExternalInput").ap()
    wq_d = nc.dram_tensor("wq", [C2, D], bf16, kind="ExternalInput").ap()
    wkT_d = nc.dram_tensor("wkT", [D, C2], bf16, kind="ExternalInput").ap()
    wv_d = nc.dram_tensor("wv", [C2, C2], bf16, kind="ExternalInput").ap()
    if has_bq:
        bq_d = nc.dram_tensor("bq", [1, D], bf16, kind="ExternalInput").ap()
    if has_bk:
        bk_d = nc.dram_tensor("bk", [1, D], bf16, kind="ExternalInput").ap()
    if has_bv:
        bv_d = nc.dram_tensor("bv", [1, C2], bf16, kind="ExternalInput").ap()
    out_d = nc.dram_tensor("out", [RPC, C2], f32, kind="ExternalOutput").ap()

    KQ = C2 // P   # 16 k-tiles over the 2048 contraction dim
    KD = D // P    # 8 k-tiles over the 1024 contraction dim
    NQ = D // 512  # 2 n-chunks for q
    NC_ = C2 // 512  # 4 n-chunks for p/ctx
    m_dt = bf16 if m_bf16 else f32

    with tile.TileContext(nc) as tc:
        with (
            tc.tile_pool(name="const", bufs=1) as constp,
            tc.tile_pool(name="histp", bufs=1) as histp,
            tc.tile_pool(name="wstream", bufs=2) as wsp,
            tc.tile_pool(name="work", bufs=1) as workp,
            tc.tile_pool(name="pbig", bufs=1, space="PSUM") as pbig,
            tc.tile_pool(name="ptp", bufs=2, space="PSUM") as ptp,
        ):
            ident = constp.tile([P, P], f32)
            make_identity(nc, ident)

            # ---- resident inputs ----
            hist_sb = histp.tile([P, H, C2], bf16)
            for g in range(8):
                nc.sync.dma_start(
                    hist_sb[:, 4 * g : 4 * (g + 1), :],
                    hist_d[:, 4 * g : 4 * (g + 1), :],
                )
            curT_sb = constp.tile([P, KQ, P], bf16)
            nc.sync.dma_start(
                curT_sb[:], curT_d.rearrange("(ko p) bt -> p ko bt", p=P)
            )
            cur_sb = constp.tile([P, C2], f32)
            nc.sync.dma_start(cur_sb[:], cur_d)
            conf_sb = constp.tile([P, 1], f32)
            nc.sync.dma_start(conf_sb[:], conf_d)

            # ---- q = cur_cat @ wq  -> psum_q [128, 1024] ----
            psum_q_full = pbig.tile([P, C2], f32, tag="big", name="psum_q")
            psum_q = psum_q_full[:, :D]
            for k in range(KQ):
                wq_k = wsp.tile([P, D], bf16, tag="wq")
                nc.sync.dma_start(wq_k[:], wq_d[k * P : (k + 1) * P, :])
                for n in range(NQ):
                    nc.tensor.matmul(
                        psum_q[:, n * 512 : (n + 1) * 512],
                        lhsT=curT_sb[:, k, :],
                        rhs=wq_k[:, n * 512 : (n + 1) * 512],
                        start=(k == 0),
                        stop=(k == KQ - 1) and not has_bq,
                    )
            if has_bq:
                bq_sb = constp.tile([1, D], bf16)
                nc.sync.dma_start(bq_sb[:], bq_d)
                ones1 = constp.tile([1, P], bf16)
                nc.vector.memset(ones1[:], 1.0)
                for n in range(NQ):
                    nc.tensor.matmul(
                        psum_q[:, n * 512 : (n + 1) * 512],
                        lhsT=ones1[:],
                        rhs=bq_sb[:, n * 512 : (n + 1) * 512],
                        start=False,
                        stop=(n == NQ - 1),
                    )
            q_sb = workp.tile([P, D], f32)
            nc.scalar.copy(q_sb[:], psum_q[:])

            # ---- qT via PE transposes (fp32 in/out, cast to bf16 on copy-back) ----
            qT_sb = workp.tile([P, KD, P], bf16)
            for dk in range(KD):
                pt = ptp.tile([P, P], f32, tag="tp")
                nc.tensor.transpose(pt[:], q_sb[:, dk * P : (dk + 1) * P], ident[:])
                nc.scalar.copy(qT_sb[:, dk, :], pt[:])

            # ---- p = q @ wk^T -> psum_p [128, 2048] ----
            psum_p = pbig.tile([P, C2], f32, tag="big")
            for dk in range(KD):
                wkT_k = wsp.tile([P, C2], bf16, tag="w2048")
                nc.sync.dma_start(wkT_k[:], wkT_d[dk * P : (dk + 1) * P, :])
                for n in range(NC_):
                    nc.tensor.matmul(
                        psum_p[:, n * 512 : (n + 1) * 512],
                        lhsT=qT_sb[:, dk, :],
                        rhs=wkT_k[:, n * 512 : (n + 1) * 512],
                        start=(dk == 0),
                        stop=(dk == KD - 1),
                    )
            p_sb = workp.tile([P, C2], bf16)
            nc.scalar.copy(p_sb[:], psum_p[:])

            # ---- fused online scores + weighted-mean pass ----
            # scores[:, h] = conf/sqrt(d) * sum_c hist[:,h,:] * p.
            # DVE does the elementwise product (bf16 2x); ScalarE does the
            # free-dim sum via activation(Copy, accum_out) with the confidence
            # scale folded into the per-partition activation scale, then the
            # exp. Logits are bounded (weights ~0.02, scale 1/32, conf<=1) so
            # softmax needs no max subtraction: e_h = exp(s_h) directly, and
            # m accumulates e_h-weighted history on DVE while ACT reduces
            # later heads. Normalization by 1/sum(e) happens once at the end.
            scores = workp.tile([P, H], f32)
            attn = workp.tile([P, H], f32)  # holds e_h = exp(s_h)
            Copy = mybir.ActivationFunctionType.Copy
            Exp = mybir.ActivationFunctionType.Exp
            m_sb = workp.tile([P, C2], m_dt)

            if has_bk:
                bk_rep = constp.tile([P, D], bf16)
                nc.sync.dma_start(bk_rep[:], bk_d.to_broadcast([P, D]))
                qbk_tmp = workp.tile([P, D], bf16)
                qbk = workp.tile([P, 1], f32)
                nc.vector.tensor_tensor(qbk_tmp[:], q_sb[:], bk_rep[:], mult)
                nc.scalar.activation(
                    qbk_tmp[:], qbk_tmp[:], Copy,
                    scale=conf_sb[:, 0:1], accum_out=qbk[:],
                )

            for h in range(H):
                tmp = workp.tile([P, C2], bf16, tag="ttr_tmp", bufs=3)
                nc.vector.tensor_tensor(tmp[:], hist_sb[:, h, :], p_sb[:], mult)
                nc.scalar.activation(
                    tmp[:], tmp[:], Copy,
                    scale=conf_sb[:, 0:1],
                    accum_out=scores[:, h : h + 1],
                )
                if has_bk:
                    nc.vector.tensor_tensor(
                        scores[:, h : h + 1], scores[:, h : h + 1], qbk[:], add
                    )
                # e_h = exp(s_h), tiny [128,1] ACT op
                nc.scalar.activation(
                    attn[:, h : h + 1], scores[:, h : h + 1], Exp
                )
                # m (+)= e_h * hist_h on DVE, overlapped with ACT's next reduce
                if h == 0:
                    nc.vector.tensor_scalar_mul(
                        m_sb[:], hist_sb[:, 0, :], attn[:, 0:1]
                    )
                else:
                    nc.vector.scalar_tensor_tensor(
                        out=m_sb[:],
                        in0=hist_sb[:, h, :],
                        scalar=attn[:, h : h + 1],
                        in1=m_sb[:],
                        op0=mult,
                        op1=add,
                    )

            # normalize: m *= 1/sum_h e_h, folded into the fp32 copy for mT
            ssum = workp.tile([P, 1], f32)
            nc.vector.reduce_sum(ssum[:], attn[:], axis=Ax.X)
            rec = workp.tile([P, 1], f32)
            nc.vector.reciprocal(rec[:], ssum[:])
            if m_bf16:
                m_f = workp.tile([P, C2], f32)
                nc.vector.tensor_scalar_mul(m_f[:], m_sb[:], rec[:, 0:1])
            else:
                nc.vector.tensor_scalar_mul(m_sb[:], m_sb[:], rec[:, 0:1])
                m_f = m_sb

            # ---- mT via PE transposes (fp32 in/out, cast to bf16 on copy-back) ----
            mT_sb = workp.tile([P, KQ, P], bf16)
            for ck in range(KQ):
                pt2 = ptp.tile([P, P], f32, tag="tp")
                nc.tensor.transpose(pt2[:], m_f[:, ck * P : (ck + 1) * P], ident[:])
                nc.scalar.copy(mT_sb[:, ck, :], pt2[:])

            # ---- ctx = m @ wv -> psum_ctx [128, 2048] ----
            psum_ctx = pbig.tile([P, C2], f32, tag="big")
            for ck in range(KQ):
                wv_k = wsp.tile([P, C2], bf16, tag="w2048")
                nc.sync.dma_start(wv_k[:], wv_d[ck * P : (ck + 1) * P, :])
                for n in range(NC_):
                    nc.tensor.matmul(
                        psum_ctx[:, n * 512 : (n + 1) * 512],
                        lhsT=mT_sb[:, ck, :],
                        rhs=wv_k[:, n * 512 : (n + 1) * 512],
                        start=(ck == 0),
                        stop=(ck == KQ - 1) and not has_bv,
                    )
            if has_bv:
                bv_sb = constp.tile([1, C2], bf16)
                nc.sync.dma_start(bv_sb[:], bv_d)
                ones1b = constp.tile([1, P], bf16)
                nc.vector.memset(ones1b[:], 1.0)
                for n in range(NC_):
                    nc.tensor.matmul(
                        psum_ctx[:, n * 512 : (n + 1) * 512],
                        lhsT=ones1b[:],
                        rhs=bv_sb[:, n * 512 : (n + 1) * 512],
                        start=False,
                        stop=(n == NC_ - 1),
                    )

            # ---- out = cur + 0.1 * ctx  (in-place into cur_sb) ----
            nc.vector.scalar_tensor_tensor(
                out=cur_sb[:],
                in0=psum_ctx[:],
                scalar=0.1,
                in1=cur_sb[:],
                op0=mult,
                op1=add,
            )
            nc.sync.dma_start(out_d, cur_sb[:])

    nc.compile()
    return nc


def _get_program(flags):
    if flags not in _CACHE:
        _CACHE[flags] = _build_program(*flags)
    return _CACHE[flags]


def kernel(**inputs) -> np.ndarray:
    hist_real = np.asarray(inputs["hist_real"], np.float32)
    hist_imag = np.asarray(inputs["hist_imag"], np.float32)
    cur_real = np.asarray(inputs["cur_real"], np.float32)
    cur_imag = np.asarray(inputs["cur_imag"], np.float32)
    confidence = np.asarray(inputs["confidence"], np.float32)
    wq = np.asarray(inputs["wq"], np.float32)
    bq = np.asarray(inputs["bq"], np.float32)
    wk = np.asarray(inputs["wk"], np.float32)
    bk = np.asarray(inputs["bk"], np.float32)
    wv = np.asarray(inputs["wv"], np.float32)
    bv = np.asarray(inputs["bv"], np.float32)

    has_bq = bool(np.any(bq))
    has_bk = bool(np.any(bk))
    has_bv = bool(np.any(bv))
    flags = (has_bq, has_bk, has_bv, False)
    nc = _get_program(flags)

    BT = B * T
    hr = hist_real.reshape(BT, H, D)
    hi = hist_imag.reshape(BT, H, D)
    cur_cat = np.concatenate(
        [cur_real.reshape(BT, D), cur_imag.reshape(BT, D)], axis=-1
    )
    conf_scaled = (confidence.reshape(BT, 1) * (D ** -0.5)).astype(np.float32)
    wq_b = np.ascontiguousarray(wq, dtype=BF16)
    wkT_b = np.ascontiguousarray(wk.T, dtype=BF16)
    wv_b = np.ascontiguousarray(wv, dtype=BF16)

    in_maps = []
    for c in range(NCORES):
        sl = slice(c * RPC, (c + 1) * RPC)
        hist_c = np.empty((RPC, H, C2), dtype=BF16)
        hist_c[:, :, :D] = hr[sl]
        hist_c[:, :, D:] = hi[sl]
        cur_c = np.ascontiguousarray(cur_cat[sl])
        m = {
            "hist": hist_c,
            "curT": np.ascontiguousarray(cur_c.T, dtype=BF16),
            "cur": cur_c,
            "conf": np.ascontiguousarray(conf_scaled[sl]),
            "wq": wq_b,
            "wkT": wkT_b,
            "wv": wv_b,
        }
        if has_bq:
            m["bq"] = np.ascontiguousarray(bq.reshape(1, D), dtype=BF16)
        if has_bk:
            m["bk"] = np.ascontiguousarray(bk.reshape(1, D), dtype=BF16)
        if has_bv:
            m["bv"] = np.ascontiguousarray(bv.reshape(1, C2), dtype=BF16)
        in_maps.append(m)

    from concourse import bass_utils

    res = bass_utils.run_bass_kernel_spmd(
        nc, in_maps, core_ids=list(range(NCORES))
    )
    out_cat = np.concatenate([r["out"] for r in res.results], axis=0)  # [1024, 2048]
    out = np.empty((BT, D), dtype=np.complex64)
    out.real = out_cat[:, :D]
    out.imag = out_cat[:, D:]
    return out.reshape(B, T, D)



# revision 20
# speedup vs baseline: 2.1628x; 2.1628x over previous
"""Trainium2 Bass kernel for nn_ComplexAttention (single-query attention over
H history slots with complex-valued channels).

Algebraic restructure (exact):
  scores[bt,h] = q[bt]·k[bt,h] = hist[bt,h]·p[bt],  p = cur_cat @ (wq @ wk^T)
  ctx[bt]      = (sum_h attn[bt,h]*hist[bt,h]) @ wv          (sum attn = 1)
  The q·bk term is constant over h and cancels in the softmax, so bk is
  dropped entirely. W1 = wq@wk^T is folded on the host (weight-only
  preprocessing), removing the q GEMM and its transposes.

Device program per core (128 (b,t) rows, hist resident as bf16 [128,H,2D]):
  p    = cur_cat @ W1       PE fp8 DoubleRow GEMM (K=256/instruction)
  per h (balanced across DVE/ACT/Pool):
    tmp_h = hist_h*p ; s_h = sum(tmp_h)    DVE TT + ACT accum-reduce, or
                                           fused DVE tensor_tensor_reduce
    e_h  = exp(conf*s_h)                   ACT (batched 4 h per op)
    m   += e_h*hist_h                      Pool STT | DVE STT' | ACT scale+DVE add
  m_norm = (m_dve+m_pool)/sum(e)           DVE
  ctx  = m_norm @ wv                       PE transposes + fp8 DoubleRow GEMM
  out  = cur + (0.1/S2)*ctx                DVE STT, then DMA out

Sharding: data-parallel over (B,T)=1024 rows, 128 rows/core, weights
replicated, no collectives.
"""

import numpy as np
import ml_dtypes

B, T, H, D = 4, 256, 32, 1024
C2 = 2 * D          # 2048
NCORES = 8
RPC = (B * T) // NCORES   # 128 rows per core
P = 128
KB = C2 // 256      # 8 double-row k-blocks

S1 = 256.0          # host prescale of W1 so fp8 stays in normal range
S2 = 64.0           # host prescale of wv

BF16 = ml_dtypes.bfloat16
FP8 = ml_dtypes.float8_e4m3

_CACHE: dict = {}


def _build_program(has_bq: bool, has_bv: bool):
    import concourse.bass as bass
    import concourse.mybir as mybir
    import concourse.tile as tile
    from concourse import bacc
    from concourse.masks import make_identity

    dt = mybir.dt
    f32, bf16, fp8 = dt.float32, dt.bfloat16, dt.float8e4
    mult = mybir.AluOpType.mult
    add = mybir.AluOpType.add
    Ax = mybir.AxisListType
    Copy = mybir.ActivationFunctionType.Copy
    Exp = mybir.ActivationFunctionType.Exp
    DR = mybir.MatmulPerfMode.DoubleRow

    nc = bacc.Bacc(
        "TRN2",
        target_bir_lowering=False,
        debug=False,
        enable_asserts=False,
        num_devices=NCORES,
    )

    hist_d = nc.dram_tensor("hist", [RPC, H, C2], bf16, kind="ExternalInput").ap()
    curT_d = nc.dram_tensor("curT8", [P, KB, 2, RPC], fp8, kind="ExternalInput").ap()
    cur_d = nc.dram_tensor("cur", [RPC, C2], f32, kind="ExternalInput").ap()
    conf_d = nc.dram_tensor("conf", [RPC, 1], f32, kind="ExternalInput").ap()
    # weights packed [p, kb, n, i, c]: rhs slice [:, kb, n, :, :] is contiguous
    w1_d = nc.dram_tensor("w18", [P, KB, 4, 2, 512], fp8, kind="ExternalInput").ap()
    wv_d = nc.dram_tensor("wv8", [P, KB, 4, 2, 512], fp8, kind="ExternalInput").ap()
    if has_bq:
        r1_d = nc.dram_tensor("r1", [1, C2], bf16, kind="ExternalInput").ap()
    if has_bv:
        bv_d = nc.dram_tensor("bvs", [1, C2], bf16, kind="ExternalInput").ap()
    out_d = nc.dram_tensor("out", [RPC, C2], f32, kind="ExternalOutput").ap()

    # per-h routing (balanced engine load; see module docstring)
    #   score: 'A' = DVE TT + ACT accum-reduce, 'B' = DVE TT + DVE reduce
    #   m scale tmp2=e_h*hist_h: 'P' on Pool tensor_scalar, 'V' on DVE
    #   (4x tensor_scalar); the h-sum accumulates on the PE (psum += I@tmp2).
    score_route = ["B" if h in (2, 7, 12, 17, 22, 27) else "A" for h in range(H)]
    m_route = [
        "P" if (h % 2 == 1 and h != 31) or h in (4, 10, 16, 22) else "V"
        for h in range(H)
    ]

    with tile.TileContext(nc) as tc:
        with (
            tc.tile_pool(name="const", bufs=1) as constp,
            tc.tile_pool(name="histp", bufs=1) as histp,
            tc.tile_pool(name="work", bufs=1) as workp,
            tc.tile_pool(name="pbig", bufs=1, space="PSUM") as pbig,
        ):
            ident = constp.tile([P, P], bf16)
            make_identity(nc, ident)

            conf_sb = constp.tile([P, 1], f32)
            cur_sb = constp.tile([P, C2], f32)
            hist_sb = histp.tile([P, H, C2], bf16)
            wv_sb = constp.tile([P, KB, 4, 2, 512], fp8)
            p_sb = workp.tile([P, C2], bf16)
            if has_bq:
                r1_sb = constp.tile([1, C2], bf16)
            if has_bv:
                bv_sb = constp.tile([1, C2], bf16)

            with tc.tile_pool(name="w1s", bufs=2) as w1p:
                # ---- DMA schedule (SP issues in program order) ----
                curT_sb = w1p.tile([P, KB, 2, RPC], fp8, tag="curT")
                nc.sync.dma_start(curT_sb[:], curT_d)
                w1_sb = []
                for kb in range(KB):
                    w1c = w1p.tile([P, 4, 2, 512], fp8, tag="w1")
                    nc.sync.dma_start(w1c[:], w1_d[:, kb, :, :, :])
                    w1_sb.append(w1c)
                nc.sync.dma_start(conf_sb[:], conf_d)
                if has_bq:
                    nc.sync.dma_start(r1_sb[:], r1_d)
                if has_bv:
                    nc.sync.dma_start(bv_sb[:], bv_d)
                for g in range(16):
                    nc.sync.dma_start(
                        hist_sb[:, 2 * g : 2 * (g + 1), :],
                        hist_d[:, 2 * g : 2 * (g + 1), :],
                    )
                nc.sync.dma_start(cur_sb[:], cur_d)
                for kb in range(KB):
                    nc.sync.dma_start(wv_sb[:, kb, :, :, :], wv_d[:, kb, :, :, :])

                # ---- p = cur_cat @ W1 (fp8 DoubleRow, K=256/instruction) ----
                psum_p = pbig.tile([P, C2], f32, tag="big", name="psum_p")
                for kb in range(KB):
                    for n in range(4):
                        nc.tensor.matmul(
                            psum_p[:, 512 * n : 512 * (n + 1)],
                            lhsT=curT_sb[:, kb, :, :],
                            rhs=w1_sb[kb][:, n, :, :],
                            start=(kb == 0),
                            stop=(kb == KB - 1) and not has_bq,
                            perf_mode=DR,
                        )
                if has_bq:
                    ones1 = constp.tile([1, P], bf16)
                    nc.vector.memset(ones1[:], 1.0)
                    for n in range(4):
                        nc.tensor.matmul(
                            psum_p[:, 512 * n : 512 * (n + 1)],
                            lhsT=ones1[:],
                            rhs=r1_sb[:, 512 * n : 512 * (n + 1)],
                            start=False,
                            stop=(n == 3),
                        )
                nc.scalar.copy(p_sb[:, :D], psum_p[:, :D])
                nc.vector.tensor_copy(p_sb[:, D:], psum_p[:, D:])

            # ---- fused scores + weighted-sum loop over H ----
            scores = workp.tile([P, H], f32)
            attn = workp.tile([P, H], f32)
            m_sb = workp.tile([P, C2], bf16)
            loop_pools = tc.tile_pool(name="tmps", bufs=3)
            tmpp = loop_pools.__enter__()
            loop_pools2 = tc.tile_pool(name="tmp2s", bufs=3)
            tmp2p = loop_pools2.__enter__()
            pm_pool = tc.tile_pool(name="pm", bufs=1, space="PSUM")
            pmp = pm_pool.__enter__()
            psum_m = pmp.tile([P, C2], f32, tag="m", name="psum_m")

            def emit_score(h):
                tmp = tmpp.tile([P, C2], bf16, tag="tmp")
                nc.vector.tensor_tensor(tmp[:], hist_sb[:, h, :], p_sb[:], mult)
                if score_route[h] == "B":
                    nc.vector.reduce_sum(
                        scores[:, h : h + 1], tmp[:], axis=Ax.X
                    )
                else:
                    nc.scalar.activation(
                        tmp[:], tmp[:], Copy, accum_out=scores[:, h : h + 1]
                    )

            def emit_exp(h0, n):
                nc.scalar.activation(
                    attn[:, h0 : h0 + n], scores[:, h0 : h0 + n], Exp,
                    scale=conf_sb[:, 0:1],
                )

            def emit_m(h):
                e_h = attn[:, h : h + 1]
                t2 = tmp2p.tile([P, C2], bf16, tag="tmp2")
                if m_route[h] == "V":
                    nc.vector.tensor_scalar_mul(t2[:], hist_sb[:, h, :], e_h)
                else:
                    nc.gpsimd.tensor_scalar_mul(t2[:], hist_sb[:, h, :], e_h)
                for n in range(4):
                    nc.tensor.matmul(
                        psum_m[:, 512 * n : 512 * (n + 1)],
                        lhsT=ident[:],
                        rhs=t2[:, 512 * n : 512 * (n + 1)],
                        start=(h == 0),
                        stop=(h == H - 1),
                    )

            for g in range(8):
                hs = range(4 * g, 4 * g + 4)
                if g < 7:
                    for h in hs:
                        emit_score(h)
                    emit_exp(4 * g, 4)
                    for h in hs:
                        emit_m(h)
                else:
                    for h in hs:
                        emit_score(h)
                        emit_exp(h, 1)
                        emit_m(h)

            loop_pools2.__exit__(None, None, None)
            loop_pools.__exit__(None, None, None)

            # ---- normalize m = psum_m / sum(e), then free psum_m ----
            ssum = workp.tile([P, 1], f32)
            nc.vector.reduce_sum(ssum[:], attn[:], axis=Ax.X)
            rec = workp.tile([P, 1], f32)
            nc.vector.reciprocal(rec[:], ssum[:])
            nc.scalar.activation(
                m_sb[:, :D], psum_m[:, :D], Copy, scale=rec[:, 0:1]
            )
            nc.vector.tensor_scalar_mul(m_sb[:, D:], psum_m[:, D:], rec[:, 0:1])
            pm_pool.__exit__(None, None, None)
            ptp_pool = tc.tile_pool(name="ptp", bufs=2, space="PSUM")
            ptp = ptp_pool.__enter__()

            # ---- mT (PE transposes, fp8 copy-back) + ctx = m @ wv ----
            mT_sb = workp.tile([P, 2 * KB, P], fp8)
            for j in range(2 * KB):
                pt = ptp.tile([P, P], bf16, tag="tp")
                nc.tensor.transpose(
                    pt[:], m_sb[:, P * j : P * (j + 1)], ident[:]
                )
                if j % 2 == 0:
                    nc.vector.tensor_copy(mT_sb[:, j, :], pt[:])
                else:
                    nc.scalar.copy(mT_sb[:, j, :], pt[:])

            psum_ctx = pbig.tile([P, C2], f32, tag="big", name="psum_ctx")
            for kb in range(KB):
                for n in range(4):
                    nc.tensor.matmul(
                        psum_ctx[:, 512 * n : 512 * (n + 1)],
                        lhsT=mT_sb[:, 2 * kb : 2 * kb + 2, :],
                        rhs=wv_sb[:, kb, n, :, :],
                        start=(kb == 0),
                        stop=(kb == KB - 1) and not has_bv,
                        perf_mode=DR,
                    )
            if has_bv:
                ones1b = constp.tile([1, P], bf16)
                nc.vector.memset(ones1b[:], 1.0)
                for n in range(4):
                    nc.tensor.matmul(
                        psum_ctx[:, 512 * n : 512 * (n + 1)],
                        lhsT=ones1b[:],
                        rhs=bv_sb[:, 512 * n : 512 * (n + 1)],
                        start=False,
                        stop=(n == 3),
                    )

            ptp_pool.__exit__(None, None, None)

            # ---- out = cur + (0.1/S2) * ctx ----
            nc.vector.scalar_tensor_tensor(
                out=cur_sb[:],
                in0=psum_ctx[:],
                scalar=0.1 / S2,
                in1=cur_sb[:],
                op0=mult,
                op1=add,
            )
            nc.sync.dma_start(out_d, cur_sb[:])

    nc.compile()
    return nc


def _get_program(flags):
    if flags not in _CACHE:
        _CACHE[flags] = _build_program(*flags)
    return _CACHE[flags]


def _pack_dr(w: np.ndarray) -> np.ndarray:
    """[2048, 2048] k-major weight -> [128, KB, 4, 2, 512] DoubleRow layout
    (arr[p, kb, n, i, c] = w[kb*256 + i*128 + p, 512*n + c]) so that every
    matmul rhs slice [:, kb, n, :, :] is contiguous."""
    return np.ascontiguousarray(
        w.reshape(KB, 2, P, 4, 512).transpose(2, 0, 3, 1, 4), dtype=FP8
    )


def kernel(**inputs) -> np.ndarray:
    hist_real = np.asarray(inputs["hist_real"], np.float32)
    hist_imag = np.asarray(inputs["hist_imag"], np.float32)
    cur_real = np.asarray(inputs["cur_real"], np.float32)
    cur_imag = np.asarray(inputs["cur_imag"], np.float32)
    confidence = np.asarray(inputs["confidence"], np.float32)
    wq = np.asarray(inputs["wq"], np.float32)
    bq = np.asarray(inputs["bq"], np.float32)
    wk = np.asarray(inputs["wk"], np.float32)
    wv = np.asarray(inputs["wv"], np.float32)
    bv = np.asarray(inputs["bv"], np.float32)
    # bk drops out: q·bk is constant over h, so the softmax is invariant to it.

    has_bq = bool(np.any(bq))
    has_bv = bool(np.any(bv))
    nc = _get_program((has_bq, has_bv))

    BT = B * T
    hr = hist_real.reshape(BT, H, D)
    hi = hist_imag.reshape(BT, H, D)
    cur_cat = np.concatenate(
        [cur_real.reshape(BT, D), cur_imag.reshape(BT, D)], axis=-1
    )
    conf_scaled = (confidence.reshape(BT, 1) / (np.sqrt(D) * S1)).astype(
        np.float32
    )
    w1_8 = _pack_dr((wq @ wk.T) * S1)
    wv_8 = _pack_dr(wv * S2)
    if has_bq:
        r1 = np.ascontiguousarray(
            ((bq @ wk.T) * S1).reshape(1, C2), dtype=BF16
        )
    if has_bv:
        bvs = np.ascontiguousarray((bv * S2).reshape(1, C2), dtype=BF16)

    in_maps = []
    for c in range(NCORES):
        sl = slice(c * RPC, (c + 1) * RPC)
        hist_c = np.empty((RPC, H, C2), dtype=BF16)
        hist_c[:, :, :D] = hr[sl]
        hist_c[:, :, D:] = hi[sl]
        cur_c = np.ascontiguousarray(cur_cat[sl])
        curT8 = np.ascontiguousarray(
            cur_c.T.reshape(KB, 2, P, RPC).transpose(2, 0, 1, 3), dtype=FP8
        )
        m = {
            "hist": hist_c,
            "curT8": curT8,
            "cur": cur_c,
            "conf": np.ascontiguousarray(conf_scaled[sl]),
            "w18": w1_8,
            "wv8": wv_8,
        }
        if has_bq:
            m["r1"] = r1
        if has_bv:
            m["bvs"] = bvs
        in_maps.append(m)

    from concourse import bass_utils

    res = bass_utils.run_bass_kernel_spmd(
        nc, in_maps, core_ids=list(range(NCORES))
    )
    out_cat = np.concatenate([r["out"] for r in res.results], axis=0)
    out = np.empty((BT, D), dtype=np.complex64)
    out.real = out_cat[:, :D]
    out.imag = out_cat[:, D:]
    return out.reshape(B, T, D)


# revision 22
# speedup vs baseline: 2.1869x; 1.0112x over previous
"""Trainium2 Bass kernel for nn_ComplexAttention (single-query attention over
H history slots with complex-valued channels).

Algebraic restructure (exact):
  scores[bt,h] = q[bt]·k[bt,h] = hist[bt,h]·p[bt],  p = cur_cat @ (wq @ wk^T)
  ctx[bt]      = (sum_h attn[bt,h]*hist[bt,h]) @ wv          (sum attn = 1)
  The q·bk term is constant over h and cancels in the softmax, so bk is
  dropped entirely. W1 = wq@wk^T is folded on the host (weight-only
  preprocessing), removing the q GEMM and its transposes.

Device program per core (128 (b,t) rows, hist resident as bf16 [128,H,2D]):
  p    = cur_cat @ W1       PE fp8 DoubleRow GEMM (K=256/instruction)
  per h (balanced across DVE/ACT/Pool):
    tmp_h = hist_h*p ; s_h = sum(tmp_h)    DVE TT + ACT accum-reduce, or
                                           fused DVE tensor_tensor_reduce
    e_h  = exp(conf*s_h)                   ACT (batched 4 h per op)
    m   += e_h*hist_h                      Pool STT | DVE STT' | ACT scale+DVE add
  m_norm = (m_dve+m_pool)/sum(e)           DVE
  ctx  = m_norm @ wv                       PE transposes + fp8 DoubleRow GEMM
  out  = cur + (0.1/S2)*ctx                DVE STT, then DMA out

Sharding: data-parallel over (B,T)=1024 rows, 128 rows/core, weights
replicated, no collectives.
"""

import numpy as np
import ml_dtypes

B, T, H, D = 4, 256, 32, 1024
C2 = 2 * D          # 2048
NCORES = 8
RPC = (B * T) // NCORES   # 128 rows per core
P = 128
KB = C2 // 256      # 8 double-row k-blocks

S1 = 256.0          # host prescale of W1 so fp8 stays in normal range
S2 = 64.0           # host prescale of wv

BF16 = ml_dtypes.bfloat16
FP8 = ml_dtypes.float8_e4m3

_CACHE: dict = {}


def _build_program(has_bq: bool, has_bv: bool):
    import concourse.bass as bass
    import concourse.mybir as mybir
    import concourse.tile as tile
    from concourse import bacc
    from concourse.masks import make_identity

    dt = mybir.dt
    f32, bf16, fp8 = dt.float32, dt.bfloat16, dt.float8e4
    mult = mybir.AluOpType.mult
    add = mybir.AluOpType.add
    Ax = mybir.AxisListType
    Copy = mybir.ActivationFunctionType.Copy
    Exp = mybir.ActivationFunctionType.Exp
    DR = mybir.MatmulPerfMode.DoubleRow

    nc = bacc.Bacc(
        "TRN2",
        target_bir_lowering=False,
        debug=False,
        enable_asserts=False,
        num_devices=NCORES,
    )

    hist_d = nc.dram_tensor("hist", [RPC, H, C2], bf16, kind="ExternalInput").ap()
    curT_d = nc.dram_tensor("curT8", [P, KB, 2, RPC], fp8, kind="ExternalInput").ap()
    cur_d = nc.dram_tensor("cur", [RPC, C2], f32, kind="ExternalInput").ap()
    conf_d = nc.dram_tensor("conf", [RPC, 1], f32, kind="ExternalInput").ap()
    # weights packed [p, kb, n, i, c]: rhs slice [:, kb, n, :, :] is contiguous
    w1_d = nc.dram_tensor("w18", [P, KB, 4, 2, 512], fp8, kind="ExternalInput").ap()
    wv_d = nc.dram_tensor("wv8", [P, KB, 4, 2, 512], fp8, kind="ExternalInput").ap()
    if has_bq:
        r1_d = nc.dram_tensor("r1", [1, C2], bf16, kind="ExternalInput").ap()
    if has_bv:
        bv_d = nc.dram_tensor("bvs", [1, C2], bf16, kind="ExternalInput").ap()
    out_d = nc.dram_tensor("out", [RPC, C2], f32, kind="ExternalOutput").ap()

    # per-h routing (balanced engine load; see module docstring)
    #   score: 'A' = DVE TT + ACT accum-reduce, 'B' = DVE TT + DVE reduce
    #   m scale tmp2=e_h*hist_h: 'P' on Pool tensor_scalar, 'V' on DVE
    #   (4x tensor_scalar); the h-sum accumulates on the PE (psum += I@tmp2).
    score_route = ["B" if h in (2, 7, 12, 17, 22, 27) else "A" for h in range(H)]
    m_route = [
        "P"
        if (h % 2 == 1 and h not in (29, 31)) or h in (4, 10, 16, 22, 26)
        else "V"
        for h in range(H)
    ]

    with tile.TileContext(nc) as tc:
        with (
            tc.tile_pool(name="const", bufs=1) as constp,
            tc.tile_pool(name="histp", bufs=1) as histp,
            tc.tile_pool(name="work", bufs=1) as workp,
            tc.tile_pool(name="pbig", bufs=1, space="PSUM") as pbig,
        ):
            ident = constp.tile([P, P], bf16)
            make_identity(nc, ident)

            conf_sb = constp.tile([P, 1], f32)
            cur_sb = constp.tile([P, C2], f32)
            hist_sb = histp.tile([P, H, C2], bf16)
            wv_sb = constp.tile([P, KB, 4, 2, 512], fp8)
            p_sb = workp.tile([P, C2], bf16)
            if has_bq:
                r1_sb = constp.tile([1, C2], bf16)
            if has_bv:
                bv_sb = constp.tile([1, C2], bf16)

            with tc.tile_pool(name="w1s", bufs=2) as w1p:
                # ---- DMA schedule (SP issues in program order) ----
                curT_sb = w1p.tile([P, KB, 2, RPC], fp8, tag="curT")
                nc.sync.dma_start(curT_sb[:], curT_d)
                w1_sb = []
                for kb in range(KB):
                    w1c = w1p.tile([P, 4, 2, 512], fp8, tag="w1")
                    nc.sync.dma_start(w1c[:], w1_d[:, kb, :, :, :])
                    w1_sb.append(w1c)
                nc.sync.dma_start(conf_sb[:], conf_d)
                if has_bq:
                    nc.sync.dma_start(r1_sb[:], r1_d)
                if has_bv:
                    nc.sync.dma_start(bv_sb[:], bv_d)
                for g in range(16):
                    nc.sync.dma_start(
                        hist_sb[:, 2 * g : 2 * (g + 1), :],
                        hist_d[:, 2 * g : 2 * (g + 1), :],
                    )
                nc.sync.dma_start(cur_sb[:], cur_d)
                for kb in range(KB):
                    nc.sync.dma_start(wv_sb[:, kb, :, :, :], wv_d[:, kb, :, :, :])

                # ---- p = cur_cat @ W1 (fp8 DoubleRow, K=256/instruction) ----
                psum_p = pbig.tile([P, C2], f32, tag="big", name="psum_p")
                for kb in range(KB):
                    for n in range(4):
                        nc.tensor.matmul(
                            psum_p[:, 512 * n : 512 * (n + 1)],
                            lhsT=curT_sb[:, kb, :, :],
                            rhs=w1_sb[kb][:, n, :, :],
                            start=(kb == 0),
                            stop=(kb == KB - 1) and not has_bq,
                            perf_mode=DR,
                        )
                if has_bq:
                    ones1 = constp.tile([1, P], bf16)
                    nc.vector.memset(ones1[:], 1.0)
                    for n in range(4):
                        nc.tensor.matmul(
                            psum_p[:, 512 * n : 512 * (n + 1)],
                            lhsT=ones1[:],
                            rhs=r1_sb[:, 512 * n : 512 * (n + 1)],
                            start=False,
                            stop=(n == 3),
                        )
                nc.scalar.copy(p_sb[:, :D], psum_p[:, :D])
                nc.vector.tensor_copy(p_sb[:, D:], psum_p[:, D:])

            # ---- fused scores + weighted-sum loop over H ----
            scores = workp.tile([P, H], f32)
            attn = workp.tile([P, H], f32)
            m_sb = workp.tile([P, C2], bf16)
            loop_pools = tc.tile_pool(name="tmps", bufs=3)
            tmpp = loop_pools.__enter__()
            loop_pools2 = tc.tile_pool(name="tmp2s", bufs=3)
            tmp2p = loop_pools2.__enter__()
            pm_pool = tc.tile_pool(name="pm", bufs=1, space="PSUM")
            pmp = pm_pool.__enter__()
            psum_m = pmp.tile([P, C2], f32, tag="m", name="psum_m")

            def emit_score(h):
                tmp = tmpp.tile([P, C2], bf16, tag="tmp")
                nc.vector.tensor_tensor(tmp[:], hist_sb[:, h, :], p_sb[:], mult)
                if score_route[h] == "B":
                    nc.vector.reduce_sum(
                        scores[:, h : h + 1], tmp[:], axis=Ax.X
                    )
                else:
                    nc.scalar.activation(
                        tmp[:], tmp[:], Copy, accum_out=scores[:, h : h + 1]
                    )

            def emit_exp(h0, n):
                nc.scalar.activation(
                    attn[:, h0 : h0 + n], scores[:, h0 : h0 + n], Exp,
                    scale=conf_sb[:, 0:1],
                )

            def emit_m(h):
                e_h = attn[:, h : h + 1]
                t2 = tmp2p.tile([P, C2], bf16, tag="tmp2")
                if m_route[h] == "V":
                    nc.vector.tensor_scalar_mul(t2[:], hist_sb[:, h, :], e_h)
                else:
                    nc.gpsimd.tensor_scalar_mul(t2[:], hist_sb[:, h, :], e_h)
                for n in range(4):
                    nc.tensor.matmul(
                        psum_m[:, 512 * n : 512 * (n + 1)],
                        lhsT=ident[:],
                        rhs=t2[:, 512 * n : 512 * (n + 1)],
                        start=(h == 0),
                        stop=(h == H - 1),
                    )

            # m-emission lags scores by one group: when DVE reaches the
            # m-scales of group g-1, exp(g-1) has already fired, so no
            # engine queue ever stalls on a not-yet-computed e_h.
            for g in range(9):
                if g < 8:
                    for h in range(4 * g, 4 * g + 4):
                        emit_score(h)
                    emit_exp(4 * g, 4)
                if g >= 1:
                    for h in range(4 * (g - 1), 4 * (g - 1) + 4):
                        emit_m(h)

            loop_pools2.__exit__(None, None, None)
            loop_pools.__exit__(None, None, None)

            # ---- normalize m = psum_m / sum(e), then free psum_m ----
            ssum = workp.tile([P, 1], f32)
            nc.vector.reduce_sum(ssum[:], attn[:], axis=Ax.X)
            rec = workp.tile([P, 1], f32)
            nc.vector.reciprocal(rec[:], ssum[:])
            nc.scalar.activation(
                m_sb[:, :D], psum_m[:, :D], Copy, scale=rec[:, 0:1]
            )
            nc.vector.tensor_scalar_mul(m_sb[:, D:], psum_m[:, D:], rec[:, 0:1])
            pm_pool.__exit__(None, None, None)
            ptp_pool = tc.tile_pool(name="ptp", bufs=2, space="PSUM")
            ptp = ptp_pool.__enter__()

            # ---- mT (PE transposes, fp8 copy-back) + ctx = m @ wv ----
            mT_sb = workp.tile([P, 2 * KB, P], fp8)
            for j in range(2 * KB):
                pt = ptp.tile([P, P], bf16, tag="tp")
                nc.tensor.transpose(
                    pt[:], m_sb[:, P * j : P * (j + 1)], ident[:]
                )
                if j % 2 == 0:
                    nc.vector.tensor_copy(mT_sb[:, j, :], pt[:])
                else:
                    nc.scalar.copy(mT_sb[:, j, :], pt[:])

            psum_ctx = pbig.tile([P, C2], f32, tag="big", name="psum_ctx")
            for kb in range(KB):
                for n in range(4):
                    nc.tensor.matmul(
                        psum_ctx[:, 512 * n : 512 * (n + 1)],
                        lhsT=mT_sb[:, 2 * kb : 2 * kb + 2, :],
                        rhs=wv_sb[:, kb, n, :, :],
                        start=(kb == 0),
                        stop=(kb == KB - 1) and not has_bv,
                        perf_mode=DR,
                    )
            if has_bv:
                ones1b = constp.tile([1, P], bf16)
                nc.vector.memset(ones1b[:], 1.0)
                for n in range(4):
                    nc.tensor.matmul(
                        psum_ctx[:, 512 * n : 512 * (n + 1)],
                        lhsT=ones1b[:],
                        rhs=bv_sb[:, 512 * n : 512 * (n + 1)],
                        start=False,
                        stop=(n == 3),
                    )

            ptp_pool.__exit__(None, None, None)

            # ---- out = cur + (0.1/S2) * ctx ----
            nc.vector.scalar_tensor_tensor(
                out=cur_sb[:],
                in0=psum_ctx[:],
                scalar=0.1 / S2,
                in1=cur_sb[:],
                op0=mult,
                op1=add,
            )
            nc.sync.dma_start(out_d, cur_sb[:])

    nc.compile()
    return nc


def _get_program(flags):
    if flags not in _CACHE:
        _CACHE[flags] = _build_program(*flags)
    return _CACHE[flags]


def _pack_dr(w: np.ndarray) -> np.ndarray:
    """[2048, 2048] k-major weight -> [128, KB, 4, 2, 512] DoubleRow layout
    (arr[p, kb, n, i, c] = w[kb*256 + i*128 + p, 512*n + c]) so that every
    matmul rhs slice [:, kb, n, :, :] is contiguous."""
    return np.ascontiguousarray(
        w.reshape(KB, 2, P, 4, 512).transpose(2, 0, 3, 1, 4), dtype=FP8
    )


def kernel(**inputs) -> np.ndarray:
    hist_real = np.asarray(inputs["hist_real"], np.float32)
    hist_imag = np.asarray(inputs["hist_imag"], np.float32)
    cur_real = np.asarray(inputs["cur_real"], np.float32)
    cur_imag = np.asarray(inputs["cur_imag"], np.float32)
    confidence = np.asarray(inputs["confidence"], np.float32)
    wq = np.asarray(inputs["wq"], np.float32)
    bq = np.asarray(inputs["bq"], np.float32)
    wk = np.asarray(inputs["wk"], np.float32)
    wv = np.asarray(inputs["wv"], np.float32)
    bv = np.asarray(inputs["bv"], np.float32)
    # bk drops out: q·bk is constant over h, so the softmax is invariant to it.

    has_bq = bool(np.any(bq))
    has_bv = bool(np.any(bv))
    nc = _get_program((has_bq, has_bv))

    BT = B * T
    hr = hist_real.reshape(BT, H, D)
    hi = hist_imag.reshape(BT, H, D)
    cur_cat = np.concatenate(
        [cur_real.reshape(BT, D), cur_imag.reshape(BT, D)], axis=-1
    )
    conf_scaled = (confidence.reshape(BT, 1) / (np.sqrt(D) * S1)).astype(
        np.float32
    )
    w1_8 = _pack_dr((wq @ wk.T) * S1)
    wv_8 = _pack_dr(wv * S2)
    if has_bq:
        r1 = np.ascontiguousarray(
            ((bq @ wk.T) * S1).reshape(1, C2), dtype=BF16
        )
    if has_bv:
        bvs = np.ascontiguousarray((bv * S2).reshape(1, C2), dtype=BF16)

    in_maps = []
    for c in range(NCORES):
        sl = slice(c * RPC, (c + 1) * RPC)
        hist_c = np.empty((RPC, H, C2), dtype=BF16)
        hist_c[:, :, :D] = hr[sl]
        hist_c[:, :, D:] = hi[sl]
        cur_c = np.ascontiguousarray(cur_cat[sl])
        curT8 = np.ascontiguousarray(
            cur_c.T.reshape(KB, 2, P, RPC).transpose(2, 0, 1, 3), dtype=FP8
        )
        m = {
            "hist": hist_c,
            "curT8": curT8,
            "cur": cur_c,
            "conf": np.ascontiguousarray(conf_scaled[sl]),
            "w18": w1_8,
            "wv8": wv_8,
        }
        if has_bq:
            m["r1"] = r1
        if has_bv:
            m["bvs"] = bvs
        in_maps.append(m)

    from concourse import bass_utils

    res = bass_utils.run_bass_kernel_spmd(
        nc, in_maps, core_ids=list(range(NCORES))
    )
    out_cat = np.concatenate([r["out"] for r in res.results], axis=0)
    out = np.empty((BT, D), dtype=np.complex64)
    out.real = out_cat[:, :D]
    out.imag = out_cat[:, D:]
    return out.reshape(B, T, D)


# revision 26
# speedup vs baseline: 2.2821x; 1.0435x over previous
"""Trainium2 Bass kernel for nn_ComplexAttention (single-query attention over
H history slots with complex-valued channels).

Algebraic restructure (exact):
  scores[bt,h] = q[bt]·k[bt,h] = hist[bt,h]·p[bt],  p = cur_cat @ (wq @ wk^T)
  ctx[bt]      = (sum_h attn[bt,h]*hist[bt,h]) @ wv          (sum attn = 1)
  The q·bk term is constant over h and cancels in the softmax, so bk is
  dropped entirely. W1 = wq@wk^T is folded on the host (weight-only
  preprocessing), removing the q GEMM and its transposes.

Device program per core (128 (b,t) rows, hist resident as bf16 [128,H,2D]):
  p    = cur_cat @ W1       PE fp8 DoubleRow GEMM (K=256/instruction)
  per h (balanced across DVE/ACT/Pool):
    tmp_h = hist_h*p ; s_h = sum(tmp_h)    DVE TT + ACT accum-reduce, or
                                           fused DVE tensor_tensor_reduce
    e_h  = exp(conf*s_h)                   ACT (batched 4 h per op)
    m   += e_h*hist_h                      Pool STT | DVE STT' | ACT scale+DVE add
  m_norm = (m_dve+m_pool)/sum(e)           DVE
  ctx  = m_norm @ wv                       PE transposes + fp8 DoubleRow GEMM
  out  = cur + (0.1/S2)*ctx                DVE STT, then DMA out

Sharding: data-parallel over (B,T)=1024 rows, 128 rows/core, weights
replicated, no collectives.
"""

import numpy as np
import ml_dtypes

B, T, H, D = 4, 256, 32, 1024
C2 = 2 * D          # 2048
NCORES = 8
RPC = (B * T) // NCORES   # 128 rows per core
P = 128
KB = C2 // 256      # 8 double-row k-blocks

S1 = 256.0          # host prescale of W1 so fp8 stays in normal range
S2 = 64.0           # host prescale of wv

BF16 = ml_dtypes.bfloat16
FP8 = ml_dtypes.float8_e4m3

_CACHE: dict = {}


def _build_program(has_bq: bool, has_bv: bool):
    import concourse.bass as bass
    import concourse.mybir as mybir
    import concourse.tile as tile
    from concourse import bacc
    from concourse.masks import make_identity

    dt = mybir.dt
    f32, bf16, fp8 = dt.float32, dt.bfloat16, dt.float8e4
    mult = mybir.AluOpType.mult
    add = mybir.AluOpType.add
    Ax = mybir.AxisListType
    Copy = mybir.ActivationFunctionType.Copy
    Exp = mybir.ActivationFunctionType.Exp
    DR = mybir.MatmulPerfMode.DoubleRow

    nc = bacc.Bacc(
        "TRN2",
        target_bir_lowering=False,
        debug=False,
        enable_asserts=False,
        num_devices=NCORES,
    )

    hist_d = nc.dram_tensor("hist", [RPC, H, C2], bf16, kind="ExternalInput").ap()
    curT_d = nc.dram_tensor("curT8", [P, KB, 2, RPC], fp8, kind="ExternalInput").ap()
    cur_d = nc.dram_tensor("cur", [RPC, C2], f32, kind="ExternalInput").ap()
    conf_d = nc.dram_tensor("conf", [RPC, 1], f32, kind="ExternalInput").ap()
    # weights packed [p, kb, n, i, c]: rhs slice [:, kb, n, :, :] is contiguous
    w1_d = nc.dram_tensor("w18", [P, KB, 4, 2, 512], fp8, kind="ExternalInput").ap()
    wv_d = nc.dram_tensor("wv8", [P, KB, 4, 2, 512], fp8, kind="ExternalInput").ap()
    if has_bq:
        r1_d = nc.dram_tensor("r1", [1, C2], bf16, kind="ExternalInput").ap()
    if has_bv:
        bv_d = nc.dram_tensor("bvs", [1, C2], bf16, kind="ExternalInput").ap()
    out_d = nc.dram_tensor("out", [RPC, C2], f32, kind="ExternalOutput").ap()

    # per-h routing (balanced engine load; see module docstring)
    #   score: 'A' = DVE TT + ACT accum-reduce, 'B' = DVE TT + DVE reduce
    #   m scale tmp2=e_h*hist_h: 'P' on Pool tensor_scalar, 'V' on DVE
    #   (4x tensor_scalar); the h-sum accumulates on the PE (psum += I@tmp2).
    score_route = ["B" if h in (2, 7, 12, 17, 22, 27) else "A" for h in range(H)]
    # 18 Pool scales, front-loaded (Pool's serial chain must start early and
    # never spill past the loop end), none in the last group.
    pool_hs = {1, 2, 3, 5, 6, 7, 9, 10, 11, 13, 14, 15, 17, 18, 21, 22, 25, 26}
    m_route = ["P" if h in pool_hs else "V" for h in range(H)]

    with tile.TileContext(nc) as tc:
        with (
            tc.tile_pool(name="const", bufs=1) as constp,
            tc.tile_pool(name="histp", bufs=1) as histp,
            tc.tile_pool(name="work", bufs=1) as workp,
            tc.tile_pool(name="pbig", bufs=1, space="PSUM") as pbig,
        ):
            ident = constp.tile([P, P], bf16)
            make_identity(nc, ident)

            conf_sb = constp.tile([P, 1], f32)
            cur_sb = constp.tile([P, C2], f32)
            hist_sb = histp.tile([P, H, C2], bf16)
            wv_sb = constp.tile([P, KB, 4, 2, 512], fp8)
            p_sb = workp.tile([P, C2], bf16)
            if has_bq:
                r1_sb = constp.tile([1, C2], bf16)
            if has_bv:
                bv_sb = constp.tile([1, C2], bf16)

            with tc.tile_pool(name="w1s", bufs=2) as w1p:
                # ---- DMA schedule (SP issues in program order) ----
                curT_sb = w1p.tile([P, KB, 2, RPC], fp8, tag="curT")
                nc.sync.dma_start(curT_sb[:], curT_d)
                w1_sb = []
                for kb in range(KB):
                    w1c = w1p.tile([P, 4, 2, 512], fp8, tag="w1")
                    nc.sync.dma_start(w1c[:], w1_d[:, kb, :, :, :])
                    w1_sb.append(w1c)
                nc.sync.dma_start(conf_sb[:], conf_d)
                if has_bq:
                    nc.sync.dma_start(r1_sb[:], r1_d)
                if has_bv:
                    nc.sync.dma_start(bv_sb[:], bv_d)
                for g in range(16):
                    nc.sync.dma_start(
                        hist_sb[:, 2 * g : 2 * (g + 1), :],
                        hist_d[:, 2 * g : 2 * (g + 1), :],
                    )
                nc.sync.dma_start(cur_sb[:], cur_d)
                for kb in range(KB):
                    nc.sync.dma_start(wv_sb[:, kb, :, :, :], wv_d[:, kb, :, :, :])

                # ---- p = cur_cat @ W1 (fp8 DoubleRow, K=256/instruction) ----
                psum_p = pbig.tile([P, C2], f32, tag="big", name="psum_p")
                for kb in range(KB):
                    for n in range(4):
                        nc.tensor.matmul(
                            psum_p[:, 512 * n : 512 * (n + 1)],
                            lhsT=curT_sb[:, kb, :, :],
                            rhs=w1_sb[kb][:, n, :, :],
                            start=(kb == 0),
                            stop=(kb == KB - 1) and not has_bq,
                            perf_mode=DR,
                        )
                if has_bq:
                    ones1 = constp.tile([1, P], bf16)
                    nc.vector.memset(ones1[:], 1.0)
                    for n in range(4):
                        nc.tensor.matmul(
                            psum_p[:, 512 * n : 512 * (n + 1)],
                            lhsT=ones1[:],
                            rhs=r1_sb[:, 512 * n : 512 * (n + 1)],
                            start=False,
                            stop=(n == 3),
                        )
                nc.scalar.copy(p_sb[:, :D], psum_p[:, :D])
                nc.vector.tensor_copy(p_sb[:, D:], psum_p[:, D:])

            # ---- fused scores + weighted-sum loop over H ----
            scores = workp.tile([P, H], f32)
            attn = workp.tile([P, H], f32)
            m_sb = workp.tile([P, C2], bf16)
            loop_pools = tc.tile_pool(name="tmps", bufs=3)
            tmpp = loop_pools.__enter__()
            loop_pools2 = tc.tile_pool(name="tmp2s", bufs=4)
            tmp2p = loop_pools2.__enter__()
            pm_pool = tc.tile_pool(name="pm", bufs=1, space="PSUM")
            pmp = pm_pool.__enter__()
            psum_m = pmp.tile([P, C2], f32, tag="m", name="psum_m")

            def emit_score(h):
                tmp = tmpp.tile([P, C2], bf16, tag="tmp")
                nc.vector.tensor_tensor(tmp[:], hist_sb[:, h, :], p_sb[:], mult)
                if score_route[h] == "B":
                    nc.vector.reduce_sum(
                        scores[:, h : h + 1], tmp[:], axis=Ax.X
                    )
                else:
                    nc.scalar.activation(
                        tmp[:], tmp[:], Copy, accum_out=scores[:, h : h + 1]
                    )

            def emit_exp(h0, n):
                nc.scalar.activation(
                    attn[:, h0 : h0 + n], scores[:, h0 : h0 + n], Exp,
                    scale=conf_sb[:, 0:1],
                )

            def emit_m(h):
                e_h = attn[:, h : h + 1]
                t2 = tmp2p.tile([P, C2], bf16, tag="tmp2")
                if m_route[h] == "V":
                    nc.vector.tensor_scalar_mul(t2[:], hist_sb[:, h, :], e_h)
                else:
                    nc.gpsimd.tensor_scalar_mul(t2[:], hist_sb[:, h, :], e_h)
                for n in range(4):
                    nc.tensor.matmul(
                        psum_m[:, 512 * n : 512 * (n + 1)],
                        lhsT=ident[:],
                        rhs=t2[:, 512 * n : 512 * (n + 1)],
                        start=(h == 0),
                        stop=(h == H - 1),
                    )

            # m-emission lags scores by one group: when DVE reaches the
            # m-scales of group g-1, exp(g-1) has already fired, so no
            # engine queue ever stalls on a not-yet-computed e_h.
            for g in range(9):
                if g < 8:
                    for h in range(4 * g, 4 * g + 4):
                        emit_score(h)
                    emit_exp(4 * g, 4)
                if g >= 1:
                    for h in range(4 * (g - 1), 4 * (g - 1) + 4):
                        emit_m(h)

            loop_pools2.__exit__(None, None, None)
            loop_pools.__exit__(None, None, None)

            # ---- normalize m = psum_m / sum(e) in 4 pipelined chunks ----
            ssum = workp.tile([P, 1], f32)
            nc.vector.reduce_sum(ssum[:], attn[:], axis=Ax.X)
            rec = workp.tile([P, 1], f32)
            nc.vector.reciprocal(rec[:], ssum[:])
            for c in range(4):
                sl = slice(512 * c, 512 * (c + 1))
                if c % 2 == 0:
                    nc.scalar.activation(
                        m_sb[:, sl], psum_m[:, sl], Copy, scale=rec[:, 0:1]
                    )
                else:
                    nc.vector.tensor_scalar_mul(
                        m_sb[:, sl], psum_m[:, sl], rec[:, 0:1]
                    )
            pm_pool.__exit__(None, None, None)
            ptp_pool = tc.tile_pool(name="ptp", bufs=2, space="PSUM")
            ptp = ptp_pool.__enter__()

            # ---- mT (PE transposes, fp8 copy-back) + ctx = m @ wv ----
            mT_sb = workp.tile([P, 2 * KB, P], fp8)
            for j in range(2 * KB):
                pt = ptp.tile([P, P], bf16, tag="tp")
                nc.tensor.transpose(
                    pt[:], m_sb[:, P * j : P * (j + 1)], ident[:]
                )
                if j % 2 == 0:
                    nc.vector.tensor_copy(mT_sb[:, j, :], pt[:])
                else:
                    nc.scalar.copy(mT_sb[:, j, :], pt[:])

            psum_ctx = pbig.tile([P, C2], f32, tag="big", name="psum_ctx")
            for kb in range(KB):
                for n in range(4):
                    nc.tensor.matmul(
                        psum_ctx[:, 512 * n : 512 * (n + 1)],
                        lhsT=mT_sb[:, 2 * kb : 2 * kb + 2, :],
                        rhs=wv_sb[:, kb, n, :, :],
                        start=(kb == 0),
                        stop=(kb == KB - 1) and not has_bv,
                        perf_mode=DR,
                    )
            if has_bv:
                ones1b = constp.tile([1, P], bf16)
                nc.vector.memset(ones1b[:], 1.0)
                for n in range(4):
                    nc.tensor.matmul(
                        psum_ctx[:, 512 * n : 512 * (n + 1)],
                        lhsT=ones1b[:],
                        rhs=bv_sb[:, 512 * n : 512 * (n + 1)],
                        start=False,
                        stop=(n == 3),
                    )

            ptp_pool.__exit__(None, None, None)

            # ---- out = cur + (0.1/S2) * ctx, halves overlapped with DMA ----
            for c in range(2):
                sl = slice(D * c, D * (c + 1))
                nc.vector.scalar_tensor_tensor(
                    out=cur_sb[:, sl],
                    in0=psum_ctx[:, sl],
                    scalar=0.1 / S2,
                    in1=cur_sb[:, sl],
                    op0=mult,
                    op1=add,
                )
                nc.sync.dma_start(out_d[:, sl], cur_sb[:, sl])

    nc.compile()
    return nc


def _get_program(flags):
    if flags not in _CACHE:
        _CACHE[flags] = _build_program(*flags)
    return _CACHE[flags]


def _pack_dr(w: np.ndarray) -> np.ndarray:
    """[2048, 2048] k-major weight -> [128, KB, 4, 2, 512] DoubleRow layout
    (arr[p, kb, n, i, c] = w[kb*256 + i*128 + p, 512*n + c]) so that every
    matmul rhs slice [:, kb, n, :, :] is contiguous."""
    return np.ascontiguousarray(
        w.reshape(KB, 2, P, 4, 512).transpose(2, 0, 3, 1, 4), dtype=FP8
    )


def kernel(**inputs) -> np.ndarray:
    hist_real = np.asarray(inputs["hist_real"], np.float32)
    hist_imag = np.asarray(inputs["hist_imag"], np.float32)
    cur_real = np.asarray(inputs["cur_real"], np.float32)
    cur_imag = np.asarray(inputs["cur_imag"], np.float32)
    confidence = np.asarray(inputs["confidence"], np.float32)
    wq = np.asarray(inputs["wq"], np.float32)
    bq = np.asarray(inputs["bq"], np.float32)
    wk = np.asarray(inputs["wk"], np.float32)
    wv = np.asarray(inputs["wv"], np.float32)
    bv = np.asarray(inputs["bv"], np.float32)
    # bk drops out: q·bk is constant over h, so the softmax is invariant to it.

    has_bq = bool(np.any(bq))
    has_bv = bool(np.any(bv))
    nc = _get_program((has_bq, has_bv))

    BT = B * T
    hr = hist_real.reshape(BT, H, D)
    hi = hist_imag.reshape(BT, H, D)
    cur_cat = np.concatenate(
        [cur_real.reshape(BT, D), cur_imag.reshape(BT, D)], axis=-1
    )
    conf_scaled = (confidence.reshape(BT, 1) / (np.sqrt(D) * S1)).astype(
        np.float32
    )
    w1_8 = _pack_dr((wq @ wk.T) * S1)
    wv_8 = _pack_dr(wv * S2)
    if has_bq:
        r1 = np.ascontiguousarray(
            ((bq @ wk.T) * S1).reshape(1, C2), dtype=BF16
        )
    if has_bv:
        bvs = np.ascontiguousarray((bv * S2).reshape(1, C2), dtype=BF16)

    in_maps = []
    for c in range(NCORES):
        sl = slice(c * RPC, (c + 1) * RPC)
        hist_c = np.empty((RPC, H, C2), dtype=BF16)
        hist_c[:, :, :D] = hr[sl]
        hist_c[:, :, D:] = hi[sl]
        cur_c = np.ascontiguousarray(cur_cat[sl])
        curT8 = np.ascontiguousarray(
            cur_c.T.reshape(KB, 2, P, RPC).transpose(2, 0, 1, 3), dtype=FP8
        )
        m = {
            "hist": hist_c,
            "curT8": curT8,
            "cur": cur_c,
            "conf": np.ascontiguousarray(conf_scaled[sl]),
            "w18": w1_8,
            "wv8": wv_8,
        }
        if has_bq:
            m["r1"] = r1
        if has_bv:
            m["bvs"] = bvs
        in_maps.append(m)

    from concourse import bass_utils

    res = bass_utils.run_bass_kernel_spmd(
        nc, in_maps, core_ids=list(range(NCORES))
    )
    out_cat = np.concatenate([r["out"] for r in res.results], axis=0)
    out = np.empty((BT, D), dtype=np.complex64)
    out.real = out_cat[:, :D]
    out.imag = out_cat[:, D:]
    return out.reshape(B, T, D)
